# revision 4
# baseline (speedup 1.0000x reference)
"""Trainium2 Bass kernel for BatchGATConv (GAT message passing).

Strategy (8 NeuronCores, SPMD), v2 — bf16 pipeline:
  - Edges partitioned by destination-node range: core c owns dst nodes
    [c*2500, (c+1)*2500). Host sorts edges by dst and packs them, per
    128-node "node tile", into 128-edge chunks (dstl=-1 pad slots).
  - Each core (replicated) projects all node features in bf16:
    po = ftT.T @ [W | W.attn_l | W.attn_r]  (264 cols, PSUM f32), then
    writes two bf16 DRAM tables:
      g[2n+b]   = [ft(n,b) 256 | el(n,b) 4]   (260-wide gather table)
      ert[2n+b] = [er(n,b) 4]                 (compact er table)
  - Aggregation per 128-edge chunk: one indirect gather of the bf16 row
    pair g[2*src] (520 elems, 1040B), er of the 128 owned dst nodes
    fetched once per tile (8-elem indirect rows from ert), per-edge er
    selected via a one-hot matmul whose lhsT comes from an SBUF-resident
    transposed selector table (stt, loaded once at startup - no per-chunk
    DMA), edge logits e = leaky(el_src + er_dst), ex = exp(e) in bf16,
    messages m = ft*ex, then one-hot segment-sum matmuls (lhsT = S built
    on-device via iota/is_equal) accumulating numerator + denominator in
    f32 PSUM across the tile's chunks. Per tile: out = leaky(num/denom).
  - All matmul operands are bf16 (single-pass PE streaming, 4x the fp32
    rate); accumulation stays f32 in PSUM. Tolerance is 2e-2; bf16
    pipeline lands ~1e-3.
  - Chunk gathers alternate between two SWDGE rings (qPoolDynamic/1).
"""

import numpy as np
import ml_dtypes

try:
    import concourse.bass as bass
except ImportError:  # pragma: no cover
    import sys

    sys.path.insert(0, "/opt/trn_rl_repo")
    import concourse.bass as bass

import concourse.bacc as bacc
import concourse.mybir as mybir
import concourse.tile as tile
from concourse.bass_utils import run_bass_kernel_spmd

P = 128
F32 = mybir.dt.float32
BF16 = mybir.dt.bfloat16
I32 = mybir.dt.int32
NPBF = ml_dtypes.bfloat16

# problem constants
N, B, DIN, H, D, E = 20000, 2, 128, 4, 64, 320000
NEG = 0.2
NCORES = 8
HB = H * B  # 8 logits per node/edge
HD = H * D  # 256 projected feats per (n, b)
FT = B * H * D  # 512 projected feats per node
WC = HD + H  # 260 = [ft | el] g-row width
GW = 2 * WC  # 520 = gathered row-pair width
PWC = HD + 2 * H  # 264 = [W | W.attn_l | W.attn_r] projection columns
ACA = HD + HB  # 264 = acc_a width: [messages b0 | exp sums]

GATHER_RINGS = 2  # spread chunk gathers across SWDGE rings
_RING_NAMES = ["qPoolDynamic", "qPoolDynamic1", "qPoolDynamic2", "qPoolDynamic3"]


def _host_prep(src, dst, n_nodes, n_cores):
    """Sort edges by dst; pack per (core, node-tile) into 128-edge chunks.

    Returns (K, idx_T, dstl_T, nid_T, stt_T):
      K: per-node-tile chunk count (shared across cores; program structure)
      idx_T[c]:  [P, SK] int32, gather row (= 2*src) per chunk slot
      dstl_T[c]: [P, SK] f32, dst-local index in [0,128) or -1 pad
      nid_T[c]:  [P, nt] int32, ert gather row (= 2*node) per tile partition
      stt_T[c]:  [P, SK*P] bf16, stt[dl, k*128+s] = 1 iff dstl[k,s] == dl
    """
    npc = n_nodes // n_cores
    nt = (npc + P - 1) // P
    order = np.argsort(dst, kind="stable")
    ss = np.ascontiguousarray(src[order]).astype(np.int64)
    ds = np.ascontiguousarray(dst[order]).astype(np.int64)

    lows = np.array(
        [c * npc + t * P for c in range(n_cores) for t in range(nt + 1)],
        dtype=np.int64,
    )
    lows = np.minimum(lows, n_nodes)
    bounds = np.searchsorted(ds, lows).reshape(n_cores, nt + 1)
    cnts = bounds[:, 1:] - bounds[:, :-1]  # [n_cores, nt]

    K = np.maximum(1, -(-cnts.max(axis=0) // P)).astype(np.int64)  # per tile
    SK = int(K.sum())
    offs = np.concatenate([[0], np.cumsum(K)[:-1]])

    idx_all = np.zeros((n_cores, SK, P), np.int32)
    dstl_all = np.full((n_cores, SK, P), -1.0, np.float32)
    for c in range(n_cores):
        for t in range(nt):
            e0 = bounds[c, t]
            cnt = int(cnts[c, t])
            if cnt == 0:
                continue
            s = np.arange(cnt)
            rows = offs[t] + s // P
            cols = s % P
            idx_all[c, rows, cols] = 2 * ss[e0 : e0 + cnt]
            dstl_all[c, rows, cols] = (ds[e0 : e0 + cnt] - (c * npc + t * P)).astype(
                np.float32
            )

    nid_all = np.zeros((n_cores, nt, P), np.int32)
    base = np.arange(P)
    for c in range(n_cores):
        for t in range(nt):
            nid_all[c, t] = 2 * np.minimum(c * npc + t * P + base, n_nodes - 1)

    idx_T = [np.ascontiguousarray(idx_all[c].T) for c in range(n_cores)]
    dstl_T = [np.ascontiguousarray(dstl_all[c].T) for c in range(n_cores)]
    nid_T = [np.ascontiguousarray(nid_all[c].T) for c in range(n_cores)]
    # SBUF-resident transposed one-hot selector: stt[dl, k*P+s] = 1 iff
    # edge slot s of chunk k has dst-local index dl.
    stt_T = []
    for c in range(n_cores):
        st = np.zeros((P, SK * P), np.float32)
        ch, sl = np.nonzero(dstl_all[c] >= 0)
        dl = dstl_all[c][ch, sl].astype(np.int64)
        st[dl, ch * P + sl] = 1.0
        stt_T.append(np.ascontiguousarray(st.astype(NPBF)))
    return list(map(int, K)), idx_T, dstl_T, nid_T, stt_T


def _build(n_nodes, npc, K):
    """Build the SPMD Bass program (identical for all cores)."""
    R = n_nodes * B
    RT = (R + P - 1) // P
    nt = len(K)
    SK = sum(K)

    nc = bacc.Bacc(trn_type="TRN2", num_swdge_queues=max(2, GATHER_RINGS))
    featT = nc.dram_tensor("featT", [DIN, R], BF16, kind="ExternalInput")
    wmat = nc.dram_tensor("wmat", [DIN, PWC], BF16, kind="ExternalInput")
    idxd = nc.dram_tensor("idx", [P, SK], I32, kind="ExternalInput")
    dstld = nc.dram_tensor("dstl", [P, SK], F32, kind="ExternalInput")
    nidd = nc.dram_tensor("nid", [P, nt], I32, kind="ExternalInput")
    sttd = nc.dram_tensor("stt", [P, SK * P], BF16, kind="ExternalInput")
    outd = nc.dram_tensor("out", [npc, FT], F32, kind="ExternalOutput")
    g = nc.dram_tensor("gtab", [R, WC], BF16)
    ertd = nc.dram_tensor("ert", [R, H], BF16)

    with tile.TileContext(nc) as tc:
        with (
            tc.tile_pool(name="const", bufs=1) as cp,
            tc.tile_pool(name="proj", bufs=8) as pp,
            tc.tile_pool(name="projps", bufs=2, space="PSUM") as ppp,
            tc.tile_pool(name="agg", bufs=12) as ag,
            tc.tile_pool(name="aggo", bufs=4) as og,
            tc.tile_pool(name="accps", bufs=2, space="PSUM") as psp,
            tc.tile_pool(name="smallps", bufs=2, space="PSUM") as psp1,
        ):
            # ---- resident constants ----
            w_sb = cp.tile([DIN, PWC], BF16)
            nc.sync.dma_start(w_sb[:], wmat[:])
            iota_i = cp.tile([P, P], I32)
            nc.gpsimd.iota(iota_i[:], pattern=[[1, P]], base=0, channel_multiplier=0)
            iota_b = cp.tile([P, P], BF16)
            nc.vector.tensor_copy(iota_b[:], iota_i[:])
            idx_res = cp.tile([P, SK], I32)
            nc.scalar.dma_start(idx_res[:], idxd[:])
            dstl_res = cp.tile([P, SK], F32)
            nc.scalar.dma_start(dstl_res[:], dstld[:])
            nid_res = cp.tile([P, nt], I32)
            nc.scalar.dma_start(nid_res[:], nidd[:])
            zero0 = cp.tile([P, PWC], F32)
            nc.gpsimd.memset(zero0[:], 0.0)
            zero_sb = cp.tile([P, PWC], F32)
            nc.vector.tensor_copy(zero_sb[:], zero0[:])
            # resident selector table: 85KB/partition, loaded once on the
            # (otherwise idle during projection) SWDGE ring 0.
            stt_sb = cp.tile([P, SK * P], BF16)
            NS = 8
            step = (SK * P + NS - 1) // NS
            for j in range(NS):
                j0 = j * step
                j1 = min(SK * P, j0 + step)
                nc.gpsimd.dma_start(stt_sb[:, j0:j1], sttd[:, j0:j1])

            # ---- projection: g[2n+b] = [ft | el], ert[2n+b] = [er] ----
            qs = (nc.sync, nc.scalar)
            for it in range(RT):
                r0 = it * P
                rows = min(P, R - r0)
                ftT = pp.tile([DIN, rows], BF16, tag="ftT")
                qs[it % 2].dma_start(ftT[:], featT[:, r0 : r0 + rows])
                po = ppp.tile([rows, PWC], F32, tag="po")
                nc.tensor.matmul(
                    po[:], lhsT=ftT[:], rhs=w_sb[:], start=True, stop=True
                )
                pout = pp.tile([rows, PWC], BF16, tag="pout")
                nc.vector.tensor_copy(pout[:], po[:])
                qs[(it + 1) % 2].dma_start(out=g[r0 : r0 + rows, :], in_=pout[:, 0:WC])
                qs[it % 2].dma_start(
                    out=ertd[r0 : r0 + rows, :], in_=pout[:, WC:PWC]
                )

            # ---- aggregation: per node tile, segment softmax + weighted sum ----
            off = 0
            for t in range(nt):
                tn = min(P, npc - t * P)
                # er of the tile's own 128 dst nodes: 8-elem spans (rows
                # 2n, 2n+1) from the compact ert table.
                er_t = og.tile([P, HB], BF16, tag="er_t")
                nc.gpsimd.indirect_dma_start(
                    out=er_t[:],
                    out_offset=None,
                    in_=ertd[:],
                    in_offset=bass.IndirectOffsetOnAxis(
                        ap=nid_res[:, t : t + 1], axis=0
                    ),
                )
                acc_a = psp.tile([P, ACA], F32, tag="acca")
                acc_b = psp.tile([P, HD], F32, tag="accb")
                # zero-clear via DVE so PSUM bank-WAW/WAR waits stay off the
                # accumulating matmuls.
                nc.vector.tensor_copy(acc_a[:], zero_sb[:, :ACA])
                nc.vector.tensor_copy(acc_b[:], zero_sb[:, :HD])
                for k in range(K[t]):
                    col = off + k
                    gt = ag.tile([P, B, WC], BF16, tag="gt")
                    gi = nc.gpsimd.indirect_dma_start(
                        out=gt[:].rearrange("p b c -> p (b c)"),
                        out_offset=None,
                        in_=g[:],
                        in_offset=bass.IndirectOffsetOnAxis(
                            ap=idx_res[:, col : col + 1], axis=0
                        ),
                    )
                    if GATHER_RINGS > 1:
                        gi.ins.queue = _RING_NAMES[col % GATHER_RINGS]
                    # S[slot, dl] = 1 iff this slot's dst-local == dl
                    S = ag.tile([P, P], BF16, tag="S")
                    nc.vector.tensor_scalar(
                        out=S[:],
                        in0=iota_b[:],
                        scalar1=dstl_res[:, col : col + 1],
                        scalar2=None,
                        op0=mybir.AluOpType.is_equal,
                    )
                    # er_dst per slot via resident transposed selector
                    eep = psp1.tile([P, HB], F32, tag="eep")
                    nc.tensor.matmul(
                        eep[:],
                        lhsT=stt_sb[:, col * P : (col + 1) * P],
                        rhs=er_t[:],
                        start=True,
                        stop=True,
                    )
                    lg = ag.tile([P, B, H], F32, tag="lg")
                    nc.vector.tensor_tensor(
                        out=lg[:],
                        in0=gt[:, :, HD:WC],
                        in1=eep[:].rearrange("p (b h) -> p b h", b=B),
                        op=mybir.AluOpType.add,
                    )
                    l1 = ag.tile([P, HB], F32, tag="l1")
                    nc.vector.scalar_tensor_tensor(
                        out=l1[:].rearrange("p (b h) -> p b h", b=B),
                        in0=lg[:],
                        scalar=NEG,
                        in1=lg[:],
                        op0=mybir.AluOpType.mult,
                        op1=mybir.AluOpType.max,
                    )
                    # m_ext = [m(b0) 256 | exs 8][m(b1) 256 | unused 8]
                    m_ext = ag.tile([P, B, ACA], BF16, tag="m")
                    nc.scalar.activation(
                        m_ext[:, 0, HD:ACA], l1[:],
                        mybir.ActivationFunctionType.Exp,
                    )
                    exs_v = m_ext[:, 0, HD:ACA]
                    nc.vector.tensor_tensor(
                        out=m_ext[:, :, 0:HD].rearrange("p b (h d) -> p b h d", d=D),
                        in0=gt[:, :, 0:HD].rearrange("p b (h d) -> p b h d", d=D),
                        in1=exs_v.rearrange("p (b h) -> p b h", b=B)[:, :, :, None]
                        .to_broadcast([P, B, H, D]),
                        op=mybir.AluOpType.mult,
                    )
                    nc.tensor.matmul(
                        acc_a[:],
                        lhsT=S[:],
                        rhs=m_ext[:, 0, :],
                        start=False,
                        stop=(k == K[t] - 1),
                        skip_group_check=True,
                    )
                    nc.tensor.matmul(
                        acc_b[:],
                        lhsT=S[:],
                        rhs=m_ext[:, 1, 0:HD],
                        start=False,
                        stop=(k == K[t] - 1),
                        skip_group_check=True,
                    )
                off += K[t]
                dsum = og.tile([P, HB], F32, tag="dsum")
                nc.vector.tensor_scalar_add(dsum[:], acc_a[:, HD:ACA], 1e-30)
                rcp = og.tile([P, HB, 1], F32, tag="rcp")
                nc.vector.reciprocal(rcp[:, :, 0], dsum[:])
                o1 = og.tile([P, HB, D], F32, tag="o1")
                nc.vector.tensor_tensor(
                    out=o1[:, 0:H, :],
                    in0=acc_a[:, 0:HD].rearrange("p (h d) -> p h d", d=D),
                    in1=rcp[:, 0:H].to_broadcast([P, H, D]),
                    op=mybir.AluOpType.mult,
                )
                nc.vector.tensor_tensor(
                    out=o1[:, H:HB, :],
                    in0=acc_b[:].rearrange("p (h d) -> p h d", d=D),
                    in1=rcp[:, H:HB].to_broadcast([P, H, D]),
                    op=mybir.AluOpType.mult,
                )
                o3 = og.tile([P, FT], F32, tag="o3")
                nc.vector.scalar_tensor_tensor(
                    out=o3[:].rearrange("p (h d) -> p h d", d=D),
                    in0=o1[:],
                    scalar=NEG,
                    in1=o1[:],
                    op0=mybir.AluOpType.mult,
                    op1=mybir.AluOpType.max,
                )
                qs[t % 2].dma_start(out=outd[t * P : t * P + tn, :], in_=o3[:tn, :])

    nc.compile()
    return nc


def _make_inputs(feat, W, attn_l, attn_r, src, dst, n_nodes, n_cores):
    feat = np.asarray(feat, dtype=np.float32)
    W = np.asarray(W, dtype=np.float32)
    attn_l = np.asarray(attn_l, dtype=np.float32)
    attn_r = np.asarray(attn_r, dtype=np.float32)
    src = np.asarray(src)
    dst = np.asarray(dst)

    featT = np.ascontiguousarray(
        feat.reshape(n_nodes * B, DIN).T.astype(NPBF)
    )
    Wl = (W.reshape(DIN, H, D) * attn_l[None]).sum(-1).astype(np.float32)
    Wr = (W.reshape(DIN, H, D) * attn_r[None]).sum(-1).astype(np.float32)
    wmat = np.ascontiguousarray(np.concatenate([W, Wl, Wr], axis=1).astype(NPBF))

    K, idx_T, dstl_T, nid_T, stt_T = _host_prep(src, dst, n_nodes, n_cores)
    in_maps = [
        {
            "featT": featT,
            "wmat": wmat,
            "idx": idx_T[c],
            "dstl": dstl_T[c],
            "nid": nid_T[c],
            "stt": stt_T[c],
        }
        for c in range(n_cores)
    ]
    return K, in_maps


_CACHE = {}


def kernel(feat, W, attn_l, attn_r, src, dst):
    K, in_maps = _make_inputs(feat, W, attn_l, attn_r, src, dst, N, NCORES)
    key = tuple(K)
    if key not in _CACHE:
        _CACHE[key] = _build(N, N // NCORES, K)
    nc = _CACHE[key]
    res = run_bass_kernel_spmd(nc, in_maps, list(range(NCORES))).results
    out = np.concatenate([res[c]["out"] for c in range(NCORES)], axis=0)
    return np.ascontiguousarray(out.reshape(N, B, H, D))


if __name__ == "__main__":
    rng = np.random.default_rng(0)
    feat = rng.standard_normal((N, B, DIN), dtype=np.float32)
    W = rng.standard_normal((DIN, H * D), dtype=np.float32) / np.sqrt(DIN)
    al = rng.standard_normal((H, D), dtype=np.float32) * 0.1
    ar = rng.standard_normal((H, D), dtype=np.float32) * 0.1
    src = rng.integers(0, N, E).astype(np.int32)
    dst = rng.integers(0, N, E).astype(np.int32)
    out = kernel(feat=feat, W=W, attn_l=al, attn_r=ar, src=src, dst=dst)
    print(out.shape, out.dtype, np.abs(out).mean())
